# revision 1
# baseline (speedup 1.0000x reference)
"""Causal multi-head attention (16 heads, d_model 1024, seq 4096) on 8 trn2 cores.

Sharding: tensor-parallel over heads — each core owns 2 heads (a 128-wide
slice of the QKV projections and the matching 128-row slice of Wo) and
produces a partial output; the host sums the 8 partials and adds the bias.

Datatypes: x/W/q/k/ctx in bf16 (PE runs bf16 at 1 col/cycle, same as fp32r,
but it halves DMA + SBUF); attention weights in fp8e4m3 written directly by
the ACT exp; v kept as fp8 value + fp8 residual (v ~= v8 + r8, quantization
error ~0.2%) so the AV matmuls run in DoubleRow perf mode: each matmul
contracts TWO j-tiles at 0.5 cycles/row, so the two-term AV still costs half
of a bf16 AV. PSUM accumulation is fp32 throughout. Softmax numerator and
denominator use the same fp8 weights (ones-column trick) so the weight
quantization largely cancels in the ratio. Measured end-to-end absmax
rel err ~1.2e-2 (budget 2e-2).

Per-core kernel (SPMD, inputs differ per core):
  - qT/kT per head via matmul(lhsT=W_chunk, rhs=xT_chunk) -> [64, n] bf16
  - v j-tile PAIRS [128, 2, 65] fp8 (+ residual pairs with a zero 65th
    column) recovered from vT by PE transposes; the ones plane-column makes
    the attention-weight sums fall out of the ctx matmul
  - scores^T for a j-tile pair = two matmuls into one PSUM [128, 2, 512]
    (diagonal tiles narrowed to the causally live q-span), one exp per
    (head, pair) on ACT straight out of PSUM -> fp8 wt; fully-masked spans
    memset to 0 on DVE; causal mask via a [128,128] affine_select on the
    diagonal subtile (fill 0 post-exp)
  - ctx^T accumulated in PSUM over pairs via fp8 DoubleRow matmuls
    (v8 then r8 term); normalized by the ones-row sums via a K=1
    outer-product broadcast matmul + one DVE multiply
  - out chunk = ctx^T-as-lhsT @ Wo-slice (bf16), evicted to SBUF fp32,
    DMA'd out
"""

import jax
import numpy as np
from jax.experimental.shard_map import shard_map
from jax.sharding import Mesh, NamedSharding, PartitionSpec

import concourse.bacc as bacc
import concourse.mybir as mybir
import concourse.tile as tile
from concourse.masks import make_identity

P = 128
N = 4096
D = 1024
HD = 64  # head dim
KC = 8  # k chunks of 128 over D
G = 8  # n/q chunks of 512
QC = 512  # q chunk width
NJ = 32  # j tiles of 128
NPR = 16  # j-tile pairs
HDP = 128  # padded v-tile columns: 64 v + ones col + 63 zero pad (DoubleRow
           # ldweights requires a full 128-column weight tile)
F32 = mybir.dt.float32
F32R = mybir.dt.float32r
BF16 = mybir.dt.bfloat16
FP8 = mybir.dt.float8e4
EXP = mybir.ActivationFunctionType.Exp
MULT = mybir.AluOpType.mult
SUB = mybir.AluOpType.subtract
IS_GE = mybir.AluOpType.is_ge
DR = mybir.MatmulPerfMode.DoubleRow

_CACHE = {}
PHASE_LOG = []  # (instruction ordinal, label) marks for sim analysis


def _mark(nc, label):
    name = nc.get_next_instruction_name()
    # peeking consumes a name; ordinal gap of 1 is irrelevant for ranges
    PHASE_LOG.append((int(name.split('-')[1]), label))


def build():
    nc = bacc.Bacc("TRN2", target_bir_lowering=False, debug=False, num_devices=8)

    xT = nc.dram_tensor("xT", [D, N], BF16, kind="ExternalInput")
    wq = nc.dram_tensor("wq", [P, D], BF16, kind="ExternalInput")  # [p, kc*128+d]
    wk = nc.dram_tensor("wk", [P, D], BF16, kind="ExternalInput")
    wv = nc.dram_tensor("wv", [P, D], BF16, kind="ExternalInput")
    wo = nc.dram_tensor("wo", [P, D], BF16, kind="ExternalInput")  # rows = core's d slice
    out = nc.dram_tensor("out", [N, D], F32, kind="ExternalOutput")

    xT_v = xT.rearrange("(kc p) n -> p kc n", p=P)  # [128, 8, 4096]

    with tile.TileContext(nc) as tc:
        with (
            tc.tile_pool(name="const", bufs=1) as const,
            tc.tile_pool(name="persist", bufs=1) as persist,
            tc.tile_pool(name="xt", bufs=3) as xtp,
            tc.tile_pool(name="vts", bufs=2) as vtsp,
            tc.tile_pool(name="wt", bufs=12) as wtp,
            tc.tile_pool(name="ctxs", bufs=4) as ctxsp,
            tc.tile_pool(name="outs", bufs=4) as outsp,
            tc.tile_pool(name="cps", bufs=12) as cpsp,
            tc.tile_pool(name="small", bufs=12) as small,
            tc.tile_pool(name="pproj", bufs=2, space="PSUM") as pproj,
            tc.tile_pool(name="psc", bufs=2, space="PSUM") as psc,
            tc.tile_pool(name="pctx", bufs=2, space="PSUM") as pctx,
        ):
            # ---- constants / persistent state ----
            wq_s = const.tile([P, D], BF16, tag="wq", name="wq_s")
            wk_s = const.tile([P, D], BF16, tag="wk", name="wk_s")
            wv_s = const.tile([P, D], BF16, tag="wv", name="wv_s")
            wo_s = const.tile([P, D], BF16, tag="wo", name="wo_s")

            def load_weights(w_s, w_d):
                nc.sync.dma_start(w_s[:], w_d[:])
            ones64f = const.tile([1, HD], F32, tag="ones64f", name="ones64f")
            nc.vector.memset(ones64f[:], 1.0)
            ones64 = const.tile([1, HD], F32R, tag="ones64", name="ones64")
            nc.vector.tensor_copy(ones64[:], ones64f[:])
            ident_f = const.tile([P, P], F32, tag="ident_f", name="ident_f")
            make_identity(nc, ident_f[:])
            ident = const.tile([P, P], BF16, tag="ident", name="ident")
            nc.vector.tensor_copy(ident[:], ident_f[:])
            ones_col = const.tile([P, 1], F32, tag="ones_col", name="ones_col")
            nc.vector.memset(ones_col[:], 1.0)

            # per-chunk qT/kT tiles [64, 512] per head; v pair tiles
            # [128, 2, 65] per j-tile pair (plane i = j-tile 2*pr+i), value
            # (vt, ones col) + residual (vtr, zero col)
            qT = [[persist.tile([HD, QC], BF16, tag=f"qT{h}_{g}", name=f"qT{h}_{g}") for g in range(G)] for h in range(2)]
            kT = [[persist.tile([HD, QC], BF16, tag=f"kT{h}_{g}", name=f"kT{h}_{g}") for g in range(G)] for h in range(2)]
            vt = [[persist.tile([P, 2, HDP], FP8, tag=f"vt{h}_{j}", name=f"vt{h}_{j}") for j in range(NPR)] for h in range(2)]
            vtr = [[persist.tile([P, 2, HDP], FP8, tag=f"vr{h}_{j}", name=f"vr{h}_{j}") for j in range(NPR)] for h in range(2)]
            for h in range(2):
                for pr in range(NPR):
                    for i in range(2):
                        nc.vector.tensor_copy(vt[h][pr][:, i, HD : HD + 1], ones_col[:])
                        nc.vector.memset(vt[h][pr][:, i, HD + 1 : HDP], 0.0)
                        nc.vector.memset(vtr[h][pr][:, i, HD : HDP], 0.0)

            # ---- projection of n-chunk g: fills qT/kT[*][g], vt/vtr[*][2g..2g+1] ----
            xt_tiles = {}

            def load_xt(g):
                if g >= G or g in xt_tiles:
                    return
                n0 = g * QC
                t = xtp.tile([P, KC, QC], BF16, tag="xt", name="xt")
                for half in range(2):  # two descriptors, parallel queues
                    nc.sync.dma_start(
                        t[:, half * 4 : half * 4 + 4, :],
                        xT_v[:, half * 4 : half * 4 + 4, n0 : n0 + QC],
                    )
                xt_tiles[g] = t

            def proj_units(g):
                _mark(nc, f"proj{g}")
                xt = xt_tiles.pop(g)
                load_xt(g + 2)  # two-ahead prefetch (issued during attention(g-1))
                for w_s, dst, mk in ((wq_s, qT, "q"), (wk_s, kT, "k")):
                    ps = pproj.tile([P, QC], F32, tag="pp", name="pp")
                    for kc in range(KC):
                        nc.tensor.matmul(
                            ps[:], w_s[:, kc * P : (kc + 1) * P], xt[:, kc, :],
                            start=(kc == 0), stop=(kc == KC - 1),
                        )
                        if kc == 3:  # half-unit: keep PE bursts under the
                            yield (mk + "h", g)  # scores-buffer depth
                    nc.vector.tensor_copy(dst[0][g][:], ps[0:HD, :])
                    nc.vector.tensor_copy(dst[1][g][:], ps[HD:P, :])
                    yield (mk, g)
                # vT then transpose back: vT = Wv^T x^T -> [128 d, 512 n]
                ps = pproj.tile([P, QC], F32, tag="pp", name="pp")
                for kc in range(KC):
                    nc.tensor.matmul(
                        ps[:], wv_s[:, kc * P : (kc + 1) * P], xt[:, kc, :],
                        start=(kc == 0), stop=(kc == KC - 1),
                    )
                    if kc == 3:
                        yield ("vh", g)
                vts = vtsp.tile([P, QC], BF16, tag="vts", name="vts")
                nc.vector.tensor_copy(vts[:], ps[:])
                yield ("vm", g)
                for s in range(4):
                    _mark(nc, f"projv{g}_{s}")
                    jt = g * 4 + s
                    pr, pl = jt // 2, jt % 2
                    pt = pproj.tile([P, P], BF16, tag="pp", name="pp")
                    nc.tensor.transpose(pt[:], vts[:, s * P : (s + 1) * P], ident[:])
                    for h in range(2):
                        v8 = vt[h][pr][:, pl, 0:HD]
                        nc.vector.tensor_copy(v8, pt[:, h * HD : (h + 1) * HD])
                        nc.vector.tensor_tensor(
                            out=vtr[h][pr][:, pl, 0:HD],
                            in0=pt[:, h * HD : (h + 1) * HD], in1=v8, op=SUB,
                        )
                    yield (f"v{s}", g)

            # ---- attention for q-chunk g (proj work for g+1 interleaved;
            # AV matmuls trail the exp pipeline by >=2 pairs so PE stays
            # ahead, and carry across the chunk boundary so ACT never waits
            # out an AV flush; the chunk's ctx eviction + finish work are
            # queued by a finalize callback when its last AV pops) ----
            _SENT = object()

            def attention(g, stream, pend, wrap_iter=None, wrap_done=None, defer_avs=False, hold_pairs=0, pre_force=None):
                npair = 2 * (g + 1)
                ctx = [pctx.tile([HDP, QC], F32, tag="ctx", name="ctx") for _ in range(2)]
                fq = finq_iter()
                stream.force(("q", g))
                for pr in range(npair):
                    _mark(nc, f"att{g}_p{pr}")
                    if pr == 2 * g:
                        # the diagonal pairs read this chunk's own kT slices
                        stream.force(("k", g))
                    for h in range(2):
                        sc = psc.tile([P, 2, QC], F32, tag="sc", name="sc")
                        wt = wtp.tile([P, 2, QC], FP8, tag="wt", name="wt")
                        spans = []
                        for i in range(2):
                            jt = 2 * pr + i
                            s = jt - 4 * g  # >=0 on the diagonal block
                            lo = max(s, 0) * P  # fully-masked q-span ends here
                            spans.append((jt, s, lo))
                            nc.tensor.matmul(
                                sc[:, i, lo:QC],
                                kT[h][jt // 4][:, (jt % 4) * P : (jt % 4 + 1) * P],
                                qT[h][g][:, lo:QC], start=True, stop=True,
                            )
                        if spans[0][2] == 0 and spans[1][2] == 0:
                            nc.scalar.activation(wt[:], sc[:], EXP, scale=0.125)
                        else:
                            for i, (jt, s, lo) in enumerate(spans):
                                if lo > 0:
                                    # SBUF-only write, so Pool may own it
                                    nc.gpsimd.memset(wt[:, i, 0:lo], 0.0)
                                nc.scalar.activation(
                                    wt[:, i, lo:QC], sc[:, i, lo:QC], EXP, scale=0.125
                                )
                        for i, (jt, s, lo) in enumerate(spans):
                            if s < 0:
                                continue
                            nc.gpsimd.affine_select(
                                out=wt[:, i, lo : lo + P],
                                in_=wt[:, i, lo : lo + P],
                                pattern=[[1, P]], compare_op=IS_GE, fill=0.0,
                                base=0, channel_multiplier=-1,
                            )
                        if pre_force is not None:
                            # the previous chunk's v tiles must be emitted
                            # before its carried AVs pop below — deferred to
                            # here so the drain runs after ACT has scores
                            stream.force(pre_force)
                            pre_force = None
                        fin = _finalize(g, ctx) if (h == 1 and pr == npair - 1) else None
                        pend.append((ctx, wt, h, pr, npair, fin))
                        if not defer_avs and len(pend) > 8:
                            _av(*pend.pop(0))
                        # fill PE with a proj unit from the global stream
                        # (or the held next-rep proj(0) at the wrap); once
                        # dry, pull deferred finish units instead. The first
                        # pair emits unencumbered so ACT gets fresh scores
                        # across the chunk boundary.
                        if pr == 0:
                            pass
                        elif stream.pull():
                            pass
                        elif wrap_iter is not None and pr >= hold_pairs:
                            m = next(wrap_iter, _SENT)
                            if m is _SENT:
                                next(fq, None)
                            elif wrap_done is not None:
                                wrap_done.add(m)
                        else:
                            next(fq, None)
                return pend

            def _av(ctx, wt, h, pr, npair, fin):
                nc.tensor.matmul(
                    ctx[h][:], vt[h][pr][:], wt[:],
                    start=(pr == 0), stop=False, perf_mode=DR,
                )
                nc.tensor.matmul(
                    ctx[h][:], vtr[h][pr][:], wt[:],
                    start=False, stop=(pr == npair - 1), perf_mode=DR,
                )
                if fin is not None:
                    fin()

            def _finalize(g, ctx):
                def fin():
                    fin_q.append(finish_compute(g, evict_ctx(ctx)))
                return fin

            def flush(pend):
                for p in pend:
                    _av(*p)
                pend.clear()

            # ---- normalize + output projection for q-chunk g ----
            # ctx PSUM is evicted immediately after the chunk's attention;
            # the PE/DVE compute is queued and drained into later attention
            # chunks' exp-paced gaps (or at the end).
            def evict_ctx(ctx):
                # reciprocals issue eagerly so the bc matmuls pulled later
                # never stall the in-order PE behind a fresh DVE chain
                # (Pool cannot read PSUM, so both copies stay on DVE)
                cps = [cpsp.tile([HDP, QC], F32, tag="cps", name="cps") for _ in range(2)]
                nc.vector.tensor_copy(cps[0][0 : HD + 1, :], ctx[0][0 : HD + 1, :])
                nc.vector.tensor_copy(cps[1][0 : HD + 1, :], ctx[1][0 : HD + 1, :])
                recs = []
                for h in range(2):
                    rec = small.tile([1, QC], F32R, tag="rec", name="rec")
                    with nc.allow_low_precision(reason="float32r is bit-identical to fp32"):
                        nc.vector.reciprocal(rec[:], cps[h][HD : HD + 1, :])
                    recs.append(rec)
                return cps, recs

            def finish_compute(g, cps_recs):
                cps, recs = cps_recs
                q0 = g * QC
                ctxs = ctxsp.tile([P, QC], BF16, tag="ctxs", name="ctxs")
                for h in range(2):
                    _mark(nc, f"fin{g}_n{h}")
                    bc = pproj.tile([HD, QC], F32, tag="pp", name="pp")
                    nc.tensor.matmul(bc[:], ones64[:], recs[h][:], start=True, stop=True)
                    nc.vector.tensor_tensor(
                        out=ctxs[h * HD : (h + 1) * HD, :],
                        in0=cps[h][0:HD, :], in1=bc[:], op=MULT,
                    )
                    yield
                for s in range(4):
                    _mark(nc, f"fin{g}_o{s}")
                    ob = outsp.tile([P, 2 * QC], F32, tag="ob", name="ob")
                    for oc in range(2):
                        po = pproj.tile([P, QC], F32, tag="pp", name="pp")
                        nc.tensor.matmul(
                            po[:], ctxs[:, s * P : (s + 1) * P],
                            wo_s[:, oc * QC : (oc + 1) * QC],
                            start=True, stop=True,
                        )
                        nc.vector.tensor_copy(ob[:, oc * QC : (oc + 1) * QC], po[:])
                    nc.sync.dma_start(out[q0 + s * P : q0 + (s + 1) * P, :], ob[:])
                    yield

            fin_q = []

            def finq_iter():
                while fin_q:
                    try:
                        next(fin_q[0])
                        yield
                    except StopIteration:
                        fin_q.pop(0)

            # ---- schedule: proj(0); then attn(g) with proj(g+1) interleaved ----
            import os

            from itertools import chain

            reps = int(os.environ.get("BASS_KERNEL_REPS", "1"))
            load_weights(wq_s, wq)
            t0 = xtp.tile([P, KC, QC], BF16, tag="xt", name="xt")
            for qtr in range(4):
                nc.sync.dma_start(t0[:, qtr * 2 : qtr * 2 + 2, :], xT_v[:, qtr * 2 : qtr * 2 + 2, 0:QC])
            xt_tiles[0] = t0
            load_weights(wk_s, wk)
            load_weights(wv_s, wv)
            load_xt(1)
            load_weights(wo_s, wo)
            # One flat pipeline over (rep, chunk): a single proj stream
            # (chunks 1..7 of the rep) drains one unit per (pr, h) slot into
            # whichever chunk has slots, with deadline forcing (Q before a
            # chunk starts, K before its diagonal pairs, V by its end). At
            # the rep wrap the next rep's proj(0) is a held side-iterator so
            # its qT/kT[0]/vt[0..1] overwrites land after chunk 7's reads.
            class Stream:
                def __init__(self, gens):
                    self.it = chain(*gens)
                    self.done = set()
                    self.dry = False

                def pull(self):
                    m = next(self.it, _SENT)
                    if m is _SENT:
                        self.dry = True
                        return False
                    self.done.add(m)
                    return True

                def force(self, marker):
                    while marker not in self.done and not self.dry:
                        self.pull()

            pend = []
            first = True
            for r in range(reps):
                if first:
                    load_xt(0)
                    load_xt(1)
                    stream = Stream([proj_units(g) for g in range(G)])
                    # emit chunk 0's q/k units up front; its v units
                    # interleave into attention(0) (AVs deferred so the PE
                    # order stays legal)
                    stream.force(("k", 0))
                else:
                    stream = next_stream
                for g in range(G):
                    wrap_iter = None
                    wrap_done = None
                    hold = 0
                    if g + 1 == G and r + 1 < reps:
                        load_xt(0)
                        load_xt(1)
                        # pre-run the next rep's proj(0..2) through chunk 7's
                        # exp-paced slots: early chunks have far fewer slots
                        # than proj units, so their proj work must come from
                        # here. Safe by pull order: proj(g')'s kT[g']/vt
                        # writes land after chunk 7's pair-2g' reads/pops.
                        wrap_iter = chain(*[proj_units(gg) for gg in range(3)])
                        hold = 2
                        next_stream = Stream([proj_units(gg) for gg in range(3, G)])
                        # the wrap markers satisfy the next rep's chunk-0..2
                        # deadline checks
                        wrap_done = next_stream.done
                    pend = attention(
                        g, stream, pend, wrap_iter=wrap_iter, wrap_done=wrap_done,
                        defer_avs=first, hold_pairs=hold,
                        pre_force=("v3", g - 1) if g else None,
                    )
                    if wrap_iter is not None:
                        # chunk 7's v tiles must be emitted before its carried
                        # AVs pop in the next rep's attention(0), and the held
                        # proj(0..2) before the next rep reads its tiles
                        stream.force(("v3", G - 1))
                        for m in wrap_iter:
                            wrap_done.add(m)
                    first = False
            stream.force(("v3", G - 1))
            flush(pend)
            for _ in finq_iter():
                pass

    nc.compile()
    return nc


def _get_nc():
    if "nc" not in _CACHE:
        _CACHE["nc"] = build()
    return _CACHE["nc"]


def _get_runner():
    """jit(shard_map(bass_exec)) over 8 cores, built once and cached.

    Mirrors bass2jax.run_bass_via_pjrt's multi-core path minus donation, so
    the pre-zeroed output operands stay valid and every call after the first
    reuses the compiled executable.
    """
    if "runner" in _CACHE:
        return _CACHE["runner"]
    from concourse import bass2jax

    nc = _get_nc()
    bass2jax.install_neuronx_cc_hook()
    partition_name = nc.partition_id_tensor.name if nc.partition_id_tensor else None
    in_names, out_names, out_avals, zero_outs = [], [], [], []
    for alloc in nc.m.functions[0].allocations:
        if not isinstance(alloc, mybir.MemoryLocationSet):
            continue
        name = alloc.memorylocations[0].name
        if alloc.kind == "ExternalInput":
            if name != partition_name:
                in_names.append(name)
        elif alloc.kind == "ExternalOutput":
            shape = tuple(alloc.tensor_shape)
            dtype = mybir.dt.np(alloc.dtype)
            out_names.append(name)
            out_avals.append(jax.core.ShapedArray(shape, dtype))
            zero_outs.append(np.zeros(shape, dtype))
    n_params = len(in_names)
    all_in = in_names + out_names
    if partition_name is not None:
        all_in.append(partition_name)

    def _body(*args):
        operands = list(args)
        if partition_name is not None:
            operands.append(bass2jax.partition_id_tensor())
        return tuple(
            bass2jax._bass_exec_p.bind(
                *operands,
                out_avals=tuple(out_avals),
                in_names=tuple(all_in),
                out_names=tuple(out_names),
                lowering_input_output_aliases=(),
                sim_require_finite=True,
                sim_require_nnan=True,
                nc=nc,
            )
        )

    mesh = Mesh(np.asarray(jax.devices()[:8]), ("core",))
    spec = PartitionSpec("core")
    fn = jax.jit(
        shard_map(
            _body,
            mesh=mesh,
            in_specs=(spec,) * (n_params + len(out_names)),
            out_specs=(spec,) * len(out_names),
            check_rep=False,
        ),
        keep_unused=True,
    )
    sharding = NamedSharding(mesh, spec)
    zeros_dev = [
        jax.device_put(np.concatenate([z] * 8, axis=0), sharding) for z in zero_outs
    ]
    _CACHE["runner"] = (fn, in_names, out_names, out_avals, zeros_dev, sharding)
    return _CACHE["runner"]


def run_sharded(maps):
    """Run the SPMD kernel on 8 cores; returns list of per-core output dicts."""
    fn, in_names, out_names, out_avals, zeros_dev, sharding = _get_runner()
    concat_in = [
        jax.device_put(
            np.concatenate([np.asarray(maps[c][n]) for c in range(8)], axis=0), sharding
        )
        for n in in_names
    ]
    outs = fn(*concat_in, *zeros_dev)
    return [
        {
            n: np.asarray(outs[i]).reshape(8, *out_avals[i].shape)[c]
            for i, n in enumerate(out_names)
        }
        for c in range(8)
    ]


def make_in_maps(x, Wq, Wk, Wv, Wo):
    import ml_dtypes

    bf16 = ml_dtypes.bfloat16
    x = np.asarray(x, dtype=np.float32)
    xT = np.ascontiguousarray(x.reshape(N, D).T.astype(bf16))
    maps = []
    for c in range(8):
        sl = slice(c * P, (c + 1) * P)

        def _w(W):
            # [1024, 128] slice -> [p, kc*128+d] so SBUF gets lhsT chunks directly
            Wc = np.asarray(W, dtype=np.float32)[:, sl]
            return np.ascontiguousarray(
                Wc.reshape(KC, P, P).transpose(1, 0, 2).reshape(P, D).astype(bf16)
            )

        maps.append(
            {
                "xT": xT,
                "wq": _w(Wq),
                "wk": _w(Wk),
                "wv": _w(Wv),
                "wo": np.ascontiguousarray(
                    np.asarray(Wo, dtype=np.float32)[sl, :].astype(bf16)
                ),
            }
        )
    return maps


def kernel(x, Wq, Wk, Wv, Wo, bo):
    maps = make_in_maps(x, Wq, Wk, Wv, Wo)
    res = run_sharded(maps)
    acc = res[0]["out"].astype(np.float32)
    for c in range(1, 8):
        acc = acc + res[c]["out"]
    acc = acc + np.asarray(bo, dtype=np.float32)[None, :]
    return acc[None].astype(np.float32)



# revision 3
# speedup vs baseline: 1.5066x; 1.5066x over previous
"""Causal multi-head attention (16 heads, d_model 1024, seq 4096) on 8 trn2 cores.

Sharding: tensor-parallel over heads — each core owns 2 heads (a 128-wide
slice of the QKV projections and the matching 128-row slice of Wo) and
produces a partial output; the host sums the 8 partials and adds the bias.

Core layout trick: the two heads live stacked in the partition dimension —
q2T/k2T tiles are [128, 512] with head 0 in partitions 0-63 and head 1 in
partitions 64-127. The per-j-tile score matmuls for the two heads then sit
on disjoint PE row-groups (tile_position (0,0) and (64,0) auto-derived from
the operand base partitions), so the hardware runs them CONCURRENTLY —
halving score-matmul wall time vs. issuing K=64 matmuls that idle half the
array. Both heads' scores land in one [128, 2, 512] PSUM tile (plane =
head), so ONE exp instruction per j-tile covers both heads, including on
diagonal tiles (both heads share the same causal narrowing), cutting ACT
instruction count and the diagonal split overhead.

Datatypes: everything bf16 on the PE (x/W/q/k/v/wt/ctxs); attention weights
are written bf16 directly by the ACT exp. PSUM accumulation fp32. The v
tiles carry a ones column per head (cols 64 and 129) so the softmax
denominator falls out of the AV matmul as ctx row 64.

Per-core kernel (SPMD, inputs differ per core):
  - q2T/k2T per chunk via matmul(lhsT=W_chunk, rhs=xT_chunk) -> [128, 512]
    bf16 (one PSUM eviction copy per chunk instead of two per-head copies)
  - v2[jt] = [128 keys, 130] bf16: [v_h0 (64) | ones | v_h1 (64) | ones],
    recovered from vT by PE transposes + one plane-strided DVE copy
  - scores^T per j-tile = two row-tiled matmuls into one PSUM [128, 2, 512]
    (diagonal tiles narrowed to the causally live q-span), one exp per
    j-tile on ACT straight out of PSUM -> bf16 wt; causal mask via a
    [128,128] affine_select per head on the diagonal subtile (fill 0)
  - ctx^T accumulated in PSUM over j-tiles via bf16 matmuls (M=65 per head,
    q-span narrowed on diagonal tiles); normalized by the ones-row sums via
    a K=1 outer-product broadcast matmul + one DVE multiply
  - out chunk = ctx^T-as-lhsT @ Wo-slice (bf16), evicted to SBUF fp32,
    DMA'd out
"""

import jax
import numpy as np
from jax.experimental.shard_map import shard_map
from jax.sharding import Mesh, NamedSharding, PartitionSpec

import concourse.bacc as bacc
import concourse.mybir as mybir
import concourse.tile as tile
from concourse.masks import make_identity

P = 128
N = 4096
D = 1024
HD = 64  # head dim
KC = 8  # k chunks of 128 over D
G = 8  # n/q chunks of 512
QC = 512  # q chunk width
NJ = 32  # j tiles of 128
VW = 2 * (HD + 1)  # v2 tile width: v_h0 | ones | v_h1 | ones
F32 = mybir.dt.float32
F32R = mybir.dt.float32r
BF16 = mybir.dt.bfloat16
EXP = mybir.ActivationFunctionType.Exp
MULT = mybir.AluOpType.mult
IS_GE = mybir.AluOpType.is_ge

_CACHE = {}
PHASE_LOG = []  # (instruction ordinal, label) marks for sim analysis


def _mark(nc, label):
    name = nc.get_next_instruction_name()
    # peeking consumes a name; ordinal gap of 1 is irrelevant for ranges
    PHASE_LOG.append((int(name.split('-')[1]), label))


def build():
    nc = bacc.Bacc("TRN2", target_bir_lowering=False, debug=False, num_devices=8)

    xT = nc.dram_tensor("xT", [D, N], BF16, kind="ExternalInput")
    wq = nc.dram_tensor("wq", [P, D], BF16, kind="ExternalInput")  # [p, kc*128+d]
    wk = nc.dram_tensor("wk", [P, D], BF16, kind="ExternalInput")
    wv = nc.dram_tensor("wv", [P, D], BF16, kind="ExternalInput")
    wo = nc.dram_tensor("wo", [P, D], BF16, kind="ExternalInput")  # rows = core's d slice
    out = nc.dram_tensor("out", [N, D], F32, kind="ExternalOutput")

    xT_v = xT.rearrange("(kc p) n -> p kc n", p=P)  # [128, 8, 4096]

    with tile.TileContext(nc) as tc:
        with (
            tc.tile_pool(name="const", bufs=1) as const,
            tc.tile_pool(name="persist", bufs=1) as persist,
            tc.tile_pool(name="xt", bufs=3) as xtp,
            tc.tile_pool(name="vts", bufs=2) as vtsp,
            tc.tile_pool(name="wt", bufs=12) as wtp,
            tc.tile_pool(name="ctxs", bufs=4) as ctxsp,
            tc.tile_pool(name="outs", bufs=4) as outsp,
            tc.tile_pool(name="cps", bufs=12) as cpsp,
            tc.tile_pool(name="small", bufs=12) as small,
            tc.tile_pool(name="pproj", bufs=2, space="PSUM") as pproj,
            tc.tile_pool(name="psc", bufs=2, space="PSUM") as psc,
            tc.tile_pool(name="pctx", bufs=2, space="PSUM") as pctx,
        ):
            # ---- constants / persistent state ----
            wq_s = const.tile([P, D], BF16, tag="wq", name="wq_s")
            wk_s = const.tile([P, D], BF16, tag="wk", name="wk_s")
            wv_s = const.tile([P, D], BF16, tag="wv", name="wv_s")
            wo_s = const.tile([P, D], BF16, tag="wo", name="wo_s")

            def load_weights(w_s, w_d):
                nc.sync.dma_start(w_s[:], w_d[:])
            ones64f = const.tile([1, HD], F32, tag="ones64f", name="ones64f")
            nc.vector.memset(ones64f[:], 1.0)
            ones64 = const.tile([1, HD], F32R, tag="ones64", name="ones64")
            nc.vector.tensor_copy(ones64[:], ones64f[:])
            ident_f = const.tile([P, P], F32, tag="ident_f", name="ident_f")
            make_identity(nc, ident_f[:])
            ident = const.tile([P, P], BF16, tag="ident", name="ident")
            nc.vector.tensor_copy(ident[:], ident_f[:])

            # per-chunk stacked qT/kT tiles [128, 512] (h0 parts 0-63, h1
            # parts 64-127); v2 tiles [128, 130] per j-tile with per-head
            # ones columns (written once, never overwritten by the per-rep
            # plane-strided v copy)
            q2T = [persist.tile([P, QC], BF16, tag=f"q2T{g}", name=f"q2T{g}") for g in range(G)]
            k2T = [persist.tile([P, QC], BF16, tag=f"k2T{g}", name=f"k2T{g}") for g in range(G)]
            v2 = [persist.tile([P, VW], BF16, tag=f"v2_{j}", name=f"v2_{j}") for j in range(NJ)]
            for j in range(NJ):
                nc.vector.memset(v2[j][:, HD : HD + 1], 1.0)
                nc.vector.memset(v2[j][:, VW - 1 : VW], 1.0)

            # ---- projection of n-chunk g: fills q2T/k2T[g], v2[4g..4g+3] ----
            xt_tiles = {}

            def load_xt(g):
                if g >= G or g in xt_tiles:
                    return
                n0 = g * QC
                t = xtp.tile([P, KC, QC], BF16, tag="xt", name="xt")
                for half in range(2):  # two descriptors, parallel queues
                    nc.sync.dma_start(
                        t[:, half * 4 : half * 4 + 4, :],
                        xT_v[:, half * 4 : half * 4 + 4, n0 : n0 + QC],
                    )
                xt_tiles[g] = t

            def proj_units(g):
                _mark(nc, f"proj{g}")
                xt = xt_tiles.pop(g)
                load_xt(g + 2)  # two-ahead prefetch (issued during attention(g-1))
                for w_s, dst, mk in ((wq_s, q2T, "q"), (wk_s, k2T, "k")):
                    ps = pproj.tile([P, QC], F32, tag="pp", name="pp")
                    for kc in range(KC):
                        nc.tensor.matmul(
                            ps[:], w_s[:, kc * P : (kc + 1) * P], xt[:, kc, :],
                            start=(kc == 0), stop=(kc == KC - 1),
                        )
                        if kc == 3:  # half-unit: keep PE bursts under the
                            yield (mk + "h", g)  # scores-buffer depth
                    nc.vector.tensor_copy(dst[g][:], ps[:])
                    yield (mk, g)
                # vT then transpose back: vT = Wv^T x^T -> [128 d, 512 n]
                ps = pproj.tile([P, QC], F32, tag="pp", name="pp")
                for kc in range(KC):
                    nc.tensor.matmul(
                        ps[:], wv_s[:, kc * P : (kc + 1) * P], xt[:, kc, :],
                        start=(kc == 0), stop=(kc == KC - 1),
                    )
                    if kc == 3:
                        yield ("vh", g)
                vts = vtsp.tile([P, QC], BF16, tag="vts", name="vts")
                nc.vector.tensor_copy(vts[:], ps[:])
                yield ("vm", g)
                for s in range(4):
                    _mark(nc, f"projv{g}_{s}")
                    jt = g * 4 + s
                    pt = pproj.tile([P, P], BF16, tag="pp", name="pp")
                    nc.tensor.transpose(pt[:], vts[:, s * P : (s + 1) * P], ident[:])
                    # one plane-strided copy: [128, 2, 64] src stride 64 ->
                    # dst stride 65 (skipping the ones columns)
                    dst = v2[jt][:].rearrange("p (two f) -> p two f", two=2)
                    src = pt[:].rearrange("p (two f) -> p two f", two=2)
                    nc.vector.tensor_copy(dst[:, :, 0:HD], src[:, :, 0:HD])
                    yield (f"v{s}", g)

            # ---- attention for q-chunk g (proj work for g+1 interleaved;
            # AV matmuls trail the exp pipeline so PE stays ahead, and carry
            # across the chunk boundary so ACT never waits out an AV flush;
            # the chunk's ctx eviction + finish work are queued by a
            # finalize callback when its last AV pops) ----
            _SENT = object()

            def attention(g, stream, pend, wrap_iter=None, wrap_done=None, defer_avs=False, hold_tiles=0, pre_force=None):
                njt = 4 * (g + 1)
                ctx = [pctx.tile([P, QC], F32, tag="ctx", name="ctx") for _ in range(2)]
                fq = finq_iter()
                stream.force(("q", g))
                for jt in range(njt):
                    _mark(nc, f"att{g}_j{jt}")
                    if jt == 4 * g:
                        # the diagonal tiles read this chunk's own k2T slices
                        stream.force(("k", g))
                    s = jt - 4 * g  # >=0 on the diagonal block
                    lo = max(s, 0) * P  # fully-masked q-span ends here
                    sc = psc.tile([P, 2, QC], F32, tag="sc", name="sc")
                    wt = wtp.tile([P, 2, QC], BF16, tag="wt", name="wt")
                    kt = k2T[jt // 4]
                    kcol = (jt % 4) * P
                    for h in range(2):
                        nc.tensor.matmul(
                            sc[:, h, lo:QC],
                            kt[h * HD : (h + 1) * HD, kcol : kcol + P],
                            q2T[g][h * HD : (h + 1) * HD, lo:QC],
                            start=True, stop=True,
                        )
                    nc.scalar.activation(wt[:, :, lo:QC], sc[:, :, lo:QC], EXP, scale=0.125)
                    if s >= 0:
                        for h in range(2):
                            nc.gpsimd.affine_select(
                                out=wt[:, h, lo : lo + P],
                                in_=wt[:, h, lo : lo + P],
                                pattern=[[1, P]], compare_op=IS_GE, fill=0.0,
                                base=0, channel_multiplier=-1,
                            )
                    if pre_force is not None:
                        # the previous chunk's v tiles must be emitted
                        # before its carried AVs pop below — deferred to
                        # here so the drain runs after ACT has scores
                        stream.force(pre_force)
                        pre_force = None
                    fin = _finalize(g, ctx) if (jt == njt - 1) else None
                    pend.append((ctx, wt, jt, njt, fin))
                    if not defer_avs and len(pend) > 8:
                        _av(*pend.pop(0))
                    # fill PE with a proj unit from the global stream
                    # (or the held next-rep proj(0) at the wrap); once
                    # dry, pull deferred finish units instead. The first
                    # tile emits unencumbered so ACT gets fresh scores
                    # across the chunk boundary.
                    if jt == 0:
                        pass
                    elif stream.pull():
                        pass
                    elif wrap_iter is not None and jt >= hold_tiles:
                        m = next(wrap_iter, _SENT)
                        if m is _SENT:
                            next(fq, None)
                        elif wrap_done is not None:
                            wrap_done.add(m)
                    else:
                        next(fq, None)
                return pend

            def _av(ctx, wt, jt, njt, fin):
                lo = 0 if jt < njt - 4 else (jt - (njt - 4)) * P
                for h in range(2):
                    nc.tensor.matmul(
                        ctx[h][0 : HD + 1, lo:QC],
                        v2[jt][:, h * (HD + 1) : (h + 1) * (HD + 1)],
                        wt[:, h, lo:QC],
                        start=(jt == 0), stop=(jt == njt - 1),
                    )
                if fin is not None:
                    fin()

            def _finalize(g, ctx):
                def fin():
                    fin_q.append(finish_compute(g, evict_ctx(ctx)))
                return fin

            def flush(pend):
                for p in pend:
                    _av(*p)
                pend.clear()

            # ---- normalize + output projection for q-chunk g ----
            # ctx PSUM is evicted immediately after the chunk's attention;
            # the PE/DVE compute is queued and drained into later attention
            # chunks' exp-paced gaps (or at the end).
            def evict_ctx(ctx):
                # reciprocals issue eagerly so the bc matmuls pulled later
                # never stall the in-order PE behind a fresh DVE chain
                # (Pool cannot read PSUM, so both copies stay on DVE)
                cps = [cpsp.tile([P, QC], F32, tag="cps", name="cps") for _ in range(2)]
                nc.vector.tensor_copy(cps[0][0 : HD + 1, :], ctx[0][0 : HD + 1, :])
                nc.vector.tensor_copy(cps[1][0 : HD + 1, :], ctx[1][0 : HD + 1, :])
                recs = []
                for h in range(2):
                    rec = small.tile([1, QC], F32R, tag="rec", name="rec")
                    with nc.allow_low_precision(reason="float32r is bit-identical to fp32"):
                        nc.vector.reciprocal(rec[:], cps[h][HD : HD + 1, :])
                    recs.append(rec)
                return cps, recs

            def finish_compute(g, cps_recs):
                cps, recs = cps_recs
                q0 = g * QC
                ctxs = ctxsp.tile([P, QC], BF16, tag="ctxs", name="ctxs")
                for h in range(2):
                    _mark(nc, f"fin{g}_n{h}")
                    bc = pproj.tile([HD, QC], F32, tag="pp", name="pp")
                    nc.tensor.matmul(bc[:], ones64[:], recs[h][:], start=True, stop=True)
                    nc.vector.tensor_tensor(
                        out=ctxs[h * HD : (h + 1) * HD, :],
                        in0=cps[h][0:HD, :], in1=bc[:], op=MULT,
                    )
                    yield
                for s in range(4):
                    _mark(nc, f"fin{g}_o{s}")
                    ob = outsp.tile([P, 2 * QC], F32, tag="ob", name="ob")
                    for oc in range(2):
                        po = pproj.tile([P, QC], F32, tag="pp", name="pp")
                        nc.tensor.matmul(
                            po[:], ctxs[:, s * P : (s + 1) * P],
                            wo_s[:, oc * QC : (oc + 1) * QC],
                            start=True, stop=True,
                        )
                        nc.vector.tensor_copy(ob[:, oc * QC : (oc + 1) * QC], po[:])
                    nc.sync.dma_start(out[q0 + s * P : q0 + (s + 1) * P, :], ob[:])
                    yield

            fin_q = []

            def finq_iter():
                while fin_q:
                    try:
                        next(fin_q[0])
                        yield
                    except StopIteration:
                        fin_q.pop(0)

            # ---- schedule: proj(0); then attn(g) with proj(g+1) interleaved ----
            import os

            from itertools import chain

            reps = int(os.environ.get("BASS_KERNEL_REPS", "1"))
            load_weights(wq_s, wq)
            t0 = xtp.tile([P, KC, QC], BF16, tag="xt", name="xt")
            for qtr in range(4):
                nc.sync.dma_start(t0[:, qtr * 2 : qtr * 2 + 2, :], xT_v[:, qtr * 2 : qtr * 2 + 2, 0:QC])
            xt_tiles[0] = t0
            load_weights(wk_s, wk)
            load_weights(wv_s, wv)
            load_xt(1)
            load_weights(wo_s, wo)
            # One flat pipeline over (rep, chunk): a single proj stream
            # (chunks 1..7 of the rep) drains one unit per j-tile slot into
            # whichever chunk has slots, with deadline forcing (Q before a
            # chunk starts, K before its diagonal tiles, V by its end). At
            # the rep wrap the next rep's proj(0) is a held side-iterator so
            # its q2T/k2T[0]/v2[0..3] overwrites land after chunk 7's reads.
            class Stream:
                def __init__(self, gens):
                    self.it = chain(*gens)
                    self.done = set()
                    self.dry = False

                def pull(self):
                    m = next(self.it, _SENT)
                    if m is _SENT:
                        self.dry = True
                        return False
                    self.done.add(m)
                    return True

                def force(self, marker):
                    while marker not in self.done and not self.dry:
                        self.pull()

            pend = []
            first = True
            for r in range(reps):
                if first:
                    load_xt(0)
                    load_xt(1)
                    stream = Stream([proj_units(g) for g in range(G)])
                    # emit chunk 0's q/k units up front; its v units
                    # interleave into attention(0) (AVs deferred so the PE
                    # order stays legal)
                    stream.force(("k", 0))
                else:
                    stream = next_stream
                for g in range(G):
                    wrap_iter = None
                    wrap_done = None
                    hold = 0
                    if g + 1 == G and r + 1 < reps:
                        load_xt(0)
                        load_xt(1)
                        # pre-run the next rep's proj(0..2) through chunk 7's
                        # exp-paced slots: early chunks have far fewer slots
                        # than proj units, so their proj work must come from
                        # here. Safe by pull order: proj(g')'s k2T[g']/v2
                        # writes land after chunk 7's tile-4g' reads/pops.
                        wrap_iter = chain(*[proj_units(gg) for gg in range(3)])
                        hold = 4
                        next_stream = Stream([proj_units(gg) for gg in range(3, G)])
                        # the wrap markers satisfy the next rep's chunk-0..2
                        # deadline checks
                        wrap_done = next_stream.done
                    pend = attention(
                        g, stream, pend, wrap_iter=wrap_iter, wrap_done=wrap_done,
                        defer_avs=first, hold_tiles=hold,
                        pre_force=("v3", g - 1) if g else None,
                    )
                    if wrap_iter is not None:
                        # chunk 7's v tiles must be emitted before its carried
                        # AVs pop in the next rep's attention(0), and the held
                        # proj(0..2) before the next rep reads its tiles
                        stream.force(("v3", G - 1))
                        for m in wrap_iter:
                            wrap_done.add(m)
                    first = False
            stream.force(("v3", G - 1))
            flush(pend)
            for _ in finq_iter():
                pass

    nc.compile()
    return nc


def _get_nc():
    if "nc" not in _CACHE:
        _CACHE["nc"] = build()
    return _CACHE["nc"]


def _get_runner():
    """jit(shard_map(bass_exec)) over 8 cores, built once and cached.

    Mirrors bass2jax.run_bass_via_pjrt's multi-core path minus donation, so
    the pre-zeroed output operands stay valid and every call after the first
    reuses the compiled executable.
    """
    if "runner" in _CACHE:
        return _CACHE["runner"]
    from concourse import bass2jax

    nc = _get_nc()
    bass2jax.install_neuronx_cc_hook()
    partition_name = nc.partition_id_tensor.name if nc.partition_id_tensor else None
    in_names, out_names, out_avals, zero_outs = [], [], [], []
    for alloc in nc.m.functions[0].allocations:
        if not isinstance(alloc, mybir.MemoryLocationSet):
            continue
        name = alloc.memorylocations[0].name
        if alloc.kind == "ExternalInput":
            if name != partition_name:
                in_names.append(name)
        elif alloc.kind == "ExternalOutput":
            shape = tuple(alloc.tensor_shape)
            dtype = mybir.dt.np(alloc.dtype)
            out_names.append(name)
            out_avals.append(jax.core.ShapedArray(shape, dtype))
            zero_outs.append(np.zeros(shape, dtype))
    n_params = len(in_names)
    all_in = in_names + out_names
    if partition_name is not None:
        all_in.append(partition_name)

    def _body(*args):
        operands = list(args)
        if partition_name is not None:
            operands.append(bass2jax.partition_id_tensor())
        return tuple(
            bass2jax._bass_exec_p.bind(
                *operands,
                out_avals=tuple(out_avals),
                in_names=tuple(all_in),
                out_names=tuple(out_names),
                lowering_input_output_aliases=(),
                sim_require_finite=True,
                sim_require_nnan=True,
                nc=nc,
            )
        )

    mesh = Mesh(np.asarray(jax.devices()[:8]), ("core",))
    spec = PartitionSpec("core")
    fn = jax.jit(
        shard_map(
            _body,
            mesh=mesh,
            in_specs=(spec,) * (n_params + len(out_names)),
            out_specs=(spec,) * len(out_names),
            check_rep=False,
        ),
        keep_unused=True,
    )
    sharding = NamedSharding(mesh, spec)
    zeros_dev = [
        jax.device_put(np.concatenate([z] * 8, axis=0), sharding) for z in zero_outs
    ]
    _CACHE["runner"] = (fn, in_names, out_names, out_avals, zeros_dev, sharding)
    return _CACHE["runner"]


def run_sharded(maps):
    """Run the SPMD kernel on 8 cores; returns list of per-core output dicts."""
    fn, in_names, out_names, out_avals, zeros_dev, sharding = _get_runner()
    concat_in = [
        jax.device_put(
            np.concatenate([np.asarray(maps[c][n]) for c in range(8)], axis=0), sharding
        )
        for n in in_names
    ]
    outs = fn(*concat_in, *zeros_dev)
    return [
        {
            n: np.asarray(outs[i]).reshape(8, *out_avals[i].shape)[c]
            for i, n in enumerate(out_names)
        }
        for c in range(8)
    ]


def make_in_maps(x, Wq, Wk, Wv, Wo):
    import ml_dtypes

    bf16 = ml_dtypes.bfloat16
    x = np.asarray(x, dtype=np.float32)
    xT = np.ascontiguousarray(x.reshape(N, D).T.astype(bf16))
    maps = []
    for c in range(8):
        sl = slice(c * P, (c + 1) * P)

        def _w(W):
            # [1024, 128] slice -> [p, kc*128+d] so SBUF gets lhsT chunks directly
            Wc = np.asarray(W, dtype=np.float32)[:, sl]
            return np.ascontiguousarray(
                Wc.reshape(KC, P, P).transpose(1, 0, 2).reshape(P, D).astype(bf16)
            )

        maps.append(
            {
                "xT": xT,
                "wq": _w(Wq),
                "wk": _w(Wk),
                "wv": _w(Wv),
                "wo": np.ascontiguousarray(
                    np.asarray(Wo, dtype=np.float32)[sl, :].astype(bf16)
                ),
            }
        )
    return maps


def kernel(x, Wq, Wk, Wv, Wo, bo):
    maps = make_in_maps(x, Wq, Wk, Wv, Wo)
    res = run_sharded(maps)
    acc = res[0]["out"].astype(np.float32)
    for c in range(1, 8):
        acc = acc + res[c]["out"]
    acc = acc + np.asarray(bo, dtype=np.float32)[None, :]
    return acc[None].astype(np.float32)


# revision 36
# speedup vs baseline: 1.7102x; 1.1351x over previous
"""Causal multi-head attention (16 heads, d_model 1024, seq 4096) on 8 trn2 cores.

Sharding: tensor-parallel over heads — each core owns 2 heads (a 128-wide
slice of the QKV projections and the matching 128-row slice of Wo) and
produces a partial output; the host sums the 8 partials and adds the bias.

Core layout trick: the two heads live stacked in the partition dimension —
q2T/k2T tiles are [128, 512] with head 0 in partitions 0-63 and head 1 in
partitions 64-127. The per-j-tile score matmuls for the two heads then sit
on disjoint PE row-groups (tile_position (0,0) and (64,0) auto-derived from
the operand base partitions), so the hardware runs them CONCURRENTLY —
halving score-matmul wall time vs. issuing K=64 matmuls that idle half the
array. Both heads' scores land in one [128, 2, 512] PSUM tile (plane =
head), so ONE exp instruction per j-tile covers both heads, including on
diagonal tiles (both heads share the same causal narrowing), cutting ACT
instruction count and the diagonal split overhead.

Datatypes: x/W/q/k in bf16 (projections and scores); attention weights are
written fp8e4 directly by the ACT exp; v is quantized on device as
v8 = fp8(v) plus r8 = fp8(v - v8), stored as the two DoubleRow planes of
one [128, 2, 128] weight tile per (head, j-tile) with a ones column (row
64 of ctx = softmax denominator). Each AV matmul streams its wt plane once
via a zero-stride broadcast AP and contracts v8*wt + r8*wt ~= v*wt in
DoubleRow perf mode. PSUM accumulation fp32 throughout. Measured
end-to-end absmax rel err ~1.3e-2 (budget 2e-2).

Per-core kernel (SPMD, inputs differ per core):
  - q2T/k2T per chunk -> [128, 512] bf16 stacked tiles (one PSUM eviction
    copy per chunk instead of two per-head copies)
  - scores^T per j-tile = two row-tiled matmuls (head 0 on PE rows 0-63,
    head 1 on rows 64-127, concurrent on silicon) into one PSUM
    [128, 2, 512] tile (diagonal tiles narrowed to the causally live
    q-span), ONE exp per j-tile covering both heads straight out of PSUM;
    causal mask via a [128,128] affine_select per head on the diagonal
    subtile (fill 0); fully-masked spans are never written or read
  - ctx^T accumulated in PSUM over j-tiles via the DR AV above (q-span
    narrowed on diagonal tiles); normalized by the ones-row sums via a
    K=1 outer-product broadcast matmul + one DVE multiply
  - out chunk = ctx^T-as-lhsT @ Wo-slice (bf16), evicted to SBUF bf16,
    one strided DMA per chunk straight from the [128, 4, 1024] staging
    tile
"""

import jax
import numpy as np
from jax.experimental.shard_map import shard_map
from jax.sharding import Mesh, NamedSharding, PartitionSpec

import concourse.bacc as bacc
import concourse.mybir as mybir
import concourse.tile as tile
from concourse.masks import make_identity

P = 128
N = 4096
D = 1024
HD = 64  # head dim
KC = 8  # k chunks of 128 over D
G = 8  # n/q chunks of 512
QC = 512  # q chunk width
NJ = 32  # j tiles of 128
HDP = 128  # padded DR weight-tile columns (DoubleRow ldweights wants 128)
F32 = mybir.dt.float32
F32R = mybir.dt.float32r
BF16 = mybir.dt.bfloat16
FP8 = mybir.dt.float8e4
EXP = mybir.ActivationFunctionType.Exp
MULT = mybir.AluOpType.mult
SUB = mybir.AluOpType.subtract
IS_GE = mybir.AluOpType.is_ge
DR = mybir.MatmulPerfMode.DoubleRow

_CACHE = {}
PHASE_LOG = []  # (instruction ordinal, label) marks for sim analysis


def _mark(nc, label):
    name = nc.get_next_instruction_name()
    # peeking consumes a name; ordinal gap of 1 is irrelevant for ranges
    PHASE_LOG.append((int(name.split('-')[1]), label))


def build():
    nc = bacc.Bacc("TRN2", target_bir_lowering=False, debug=False, num_devices=8)

    xT = nc.dram_tensor("xT", [D, N], BF16, kind="ExternalInput")
    wq = nc.dram_tensor("wq", [P, D], BF16, kind="ExternalInput")  # [p, kc*128+d]
    wk = nc.dram_tensor("wk", [P, D], BF16, kind="ExternalInput")
    wv = nc.dram_tensor("wv", [P, D], BF16, kind="ExternalInput")
    wo = nc.dram_tensor("wo", [P, D], BF16, kind="ExternalInput")  # rows = core's d slice
    # bf16 partials halve the output DMA; the host accumulates in fp32
    out = nc.dram_tensor("out", [N, D], BF16, kind="ExternalOutput")

    xT_v = xT.rearrange("(kc p) n -> p kc n", p=P)  # [128, 8, 4096]

    with tile.TileContext(nc) as tc:
        with (
            tc.tile_pool(name="const", bufs=1) as const,
            tc.tile_pool(name="persist", bufs=1) as persist,
            tc.tile_pool(name="xt", bufs=3) as xtp,
            tc.tile_pool(name="vts", bufs=2) as vtsp,
            tc.tile_pool(name="wt", bufs=12) as wtp,
            tc.tile_pool(name="ctxs", bufs=4) as ctxsp,
            tc.tile_pool(name="outs", bufs=3) as outsp,
            tc.tile_pool(name="cps", bufs=12) as cpsp,
            tc.tile_pool(name="small", bufs=12) as small,
            tc.tile_pool(name="pproj", bufs=2, space="PSUM") as pproj,
            tc.tile_pool(name="psc", bufs=2, space="PSUM") as psc,
            tc.tile_pool(name="pctx", bufs=2, space="PSUM") as pctx,
        ):
            # ---- constants / persistent state ----
            wq_s = const.tile([P, D], BF16, tag="wq", name="wq_s")
            wk_s = const.tile([P, D], BF16, tag="wk", name="wk_s")
            wv_s = const.tile([P, D], BF16, tag="wv", name="wv_s")
            wo_s = const.tile([P, D], BF16, tag="wo", name="wo_s")

            def load_weights(w_s, w_d):
                nc.sync.dma_start(w_s[:], w_d[:])
            ones64f = const.tile([1, HD], F32, tag="ones64f", name="ones64f")
            nc.vector.memset(ones64f[:], 1.0)
            ones64 = const.tile([1, HD], F32R, tag="ones64", name="ones64")
            nc.vector.tensor_copy(ones64[:], ones64f[:])
            ident_f = const.tile([P, P], F32, tag="ident_f", name="ident_f")
            make_identity(nc, ident_f[:])
            ident = const.tile([P, P], BF16, tag="ident", name="ident")
            nc.vector.tensor_copy(ident[:], ident_f[:])

            # per-chunk stacked qT/kT tiles [128, 512] (h0 parts 0-63, h1
            # parts 64-127); v2 DoubleRow tiles [128, 2, 128] fp8 per
            # (head, j-tile): plane 0 = v8 (quantized v) + ones col 64,
            # plane 1 = r8 (v - v8 residual) + zero col — one DR matmul per
            # j-tile then contracts v8*wt + r8*wt ~= v*wt at 0.5 cyc/col.
            # The ones/zero/pad columns are written once; the per-rep v
            # prep only rewrites cols 0-63 of each plane.
            q2T = [persist.tile([P, QC], BF16, tag=f"q2T{g}", name=f"q2T{g}") for g in range(G)]
            k2T = [persist.tile([P, QC], BF16, tag=f"k2T{g}", name=f"k2T{g}") for g in range(G)]
            v2 = [[persist.tile([P, 2, HDP], FP8, tag=f"v2_{h}_{j}", name=f"v2_{h}_{j}") for j in range(NJ)] for h in range(2)]
            for h in range(2):
                for j in range(NJ):
                    nc.vector.memset(v2[h][j][:, :, HD:HDP], 0.0)
                    nc.vector.memset(v2[h][j][:, 0, HD : HD + 1], 1.0)

            # ---- projection of n-chunk g: fills q2T/k2T[g], v2[4g..4g+3] ----
            xt_tiles = {}

            def load_xt(g):
                if g >= G or g in xt_tiles:
                    return
                n0 = g * QC
                t = xtp.tile([P, KC, QC], BF16, tag="xt", name="xt")
                nc.sync.dma_start(t[:], xT_v[:, :, n0 : n0 + QC])
                xt_tiles[g] = t

            def proj_units(g):
                _mark(nc, f"proj{g}")
                xt = xt_tiles.pop(g)
                load_xt(g + 2)  # two-ahead prefetch (issued during attention(g-1))
                for w_s, dst, mk in ((wq_s, q2T, "q"), (wk_s, k2T, "k")):
                    ps = pproj.tile([P, QC], F32, tag="pp", name="pp")
                    for kc in range(KC):
                        nc.tensor.matmul(
                            ps[:], w_s[:, kc * P : (kc + 1) * P], xt[:, kc, :],
                            start=(kc == 0), stop=(kc == KC - 1),
                        )
                        if kc == 3:  # half-unit: keep PE bursts under the
                            yield (mk + "h", g)  # scores-buffer depth
                    nc.vector.tensor_copy(dst[g][:], ps[:])
                    yield (mk, g)
                # vT then transpose back: vT = Wv^T x^T -> [128 d, 512 n]
                ps = pproj.tile([P, QC], F32, tag="pp", name="pp")
                for kc in range(KC):
                    nc.tensor.matmul(
                        ps[:], wv_s[:, kc * P : (kc + 1) * P], xt[:, kc, :],
                        start=(kc == 0), stop=(kc == KC - 1),
                    )
                    if kc == 3:
                        yield ("vh", g)
                vts = vtsp.tile([P, QC], BF16, tag="vts", name="vts")
                nc.vector.tensor_copy(vts[:], ps[:])
                yield ("vm", g)
                for s in range(4):
                    _mark(nc, f"projv{g}_{s}")
                    jt = g * 4 + s
                    pt = pproj.tile([P, P], BF16, tag="pp", name="pp")
                    nc.tensor.transpose(pt[:], vts[:, s * P : (s + 1) * P], ident[:])
                    for h in range(2):
                        v8 = v2[h][jt][:, 0, 0:HD]
                        nc.vector.tensor_copy(v8, pt[:, h * HD : (h + 1) * HD])
                        nc.vector.tensor_tensor(
                            out=v2[h][jt][:, 1, 0:HD],
                            in0=pt[:, h * HD : (h + 1) * HD], in1=v8, op=SUB,
                        )
                    yield (f"v{s}", g)

            # ---- attention for q-chunk g (proj work for g+1 interleaved;
            # AV matmuls trail the exp pipeline so PE stays ahead, and carry
            # across the chunk boundary so ACT never waits out an AV flush;
            # the chunk's ctx eviction + finish work are queued by a
            # finalize callback when its last AV pops) ----
            _SENT = object()

            def attention(g, stream, pend, wrap_iter=None, wrap_done=None, defer_avs=False, hold_tiles=0, pre_force=None):
                njt = 4 * (g + 1)
                ctx = [pctx.tile([P, QC], F32, tag="ctx", name="ctx") for _ in range(2)]
                fq = finq_iter()
                stream.force(("q", g))
                for jt in range(njt):
                    _mark(nc, f"att{g}_j{jt}")
                    if jt == 4 * g:
                        # the diagonal tiles read this chunk's own k2T slices
                        stream.force(("k", g))
                    s = jt - 4 * g  # >=0 on the diagonal block
                    lo = max(s, 0) * P  # fully-masked q-span ends here
                    sc = psc.tile([P, 2, QC], F32, tag="sc", name="sc")
                    wt = wtp.tile([P, 2, QC], FP8, tag="wt", name="wt")
                    kt = k2T[jt // 4]
                    kcol = (jt % 4) * P
                    for h in range(2):
                        nc.tensor.matmul(
                            sc[:, h, lo:QC],
                            kt[h * HD : (h + 1) * HD, kcol : kcol + P],
                            q2T[g][h * HD : (h + 1) * HD, lo:QC],
                            start=True, stop=True,
                        )
                    nc.scalar.activation(wt[:, :, lo:QC], sc[:, :, lo:QC], EXP, scale=0.125)
                    if s >= 0:
                        for h in range(2):
                            nc.gpsimd.affine_select(
                                out=wt[:, h, lo : lo + P],
                                in_=wt[:, h, lo : lo + P],
                                pattern=[[1, P]], compare_op=IS_GE, fill=0.0,
                                base=0, channel_multiplier=-1,
                            )
                    if pre_force is not None:
                        # the previous chunk's v tiles must be emitted
                        # before its carried AVs pop below — deferred to
                        # here so the drain runs after ACT has scores
                        stream.force(pre_force)
                        pre_force = None
                    fin = _finalize(g, ctx) if (jt == njt - 1) else None
                    pend.append((ctx, wt, jt, njt, fin))
                    if not defer_avs and len(pend) > 8:
                        _av(*pend.pop(0))
                    # fill PE with a proj unit from the global stream
                    # (or the held next-rep proj(0) at the wrap); once
                    # dry, pull deferred finish units instead. The first
                    # tile emits unencumbered so ACT gets fresh scores
                    # across the chunk boundary.
                    if jt == 0:
                        pass
                    elif stream.pull():
                        pass
                    elif wrap_iter is not None and jt >= hold_tiles:
                        m = next(wrap_iter, _SENT)
                        if m is _SENT:
                            next(fq, None)
                        elif wrap_done is not None:
                            wrap_done.add(m)
                    else:
                        next(fq, None)
                return pend

            def _av(ctx, wt, jt, njt, fin):
                lo = 0 if jt < njt - 4 else (jt - (njt - 4)) * P
                for h in range(2):
                    # rhs: the same wt plane streamed for both DR planes
                    # (v8 then r8) via a zero-stride broadcast
                    rhs = (
                        wt[:, h, lo:QC]
                        .rearrange("p (one n) -> p one n", one=1)
                        .broadcast_to([P, 2, QC - lo])
                    )
                    nc.tensor.matmul(
                        ctx[h][:, lo:QC], v2[h][jt][:], rhs,
                        start=(jt == 0), stop=(jt == njt - 1), perf_mode=DR,
                    )
                if fin is not None:
                    fin()

            def _finalize(g, ctx):
                def fin():
                    fin_q.append(finish_compute(g, evict_ctx(ctx)))
                return fin

            def flush(pend):
                for p in pend:
                    _av(*p)
                pend.clear()

            # ---- normalize + output projection for q-chunk g ----
            # ctx PSUM is evicted immediately after the chunk's attention;
            # the PE/DVE compute is queued and drained into later attention
            # chunks' exp-paced gaps (or at the end).
            def evict_ctx(ctx):
                # reciprocals issue eagerly so the bc matmuls pulled later
                # never stall the in-order PE behind a fresh DVE chain
                # (Pool cannot read PSUM, so both copies stay on DVE)
                cps = [cpsp.tile([P, QC], F32, tag="cps", name="cps") for _ in range(2)]
                nc.vector.tensor_copy(cps[0][0 : HD + 1, :], ctx[0][0 : HD + 1, :])
                nc.vector.tensor_copy(cps[1][0 : HD + 1, :], ctx[1][0 : HD + 1, :])
                recs = []
                for h in range(2):
                    rec = small.tile([1, QC], F32R, tag="rec", name="rec")
                    with nc.allow_low_precision(reason="float32r is bit-identical to fp32"):
                        nc.vector.reciprocal(rec[:], cps[h][HD : HD + 1, :])
                    recs.append(rec)
                return cps, recs

            out_v = out.rearrange("(n s p) d -> p n s d", s=4, p=P)  # [128, 8, 4, 1024]

            def finish_compute(g, cps_recs):
                cps, recs = cps_recs
                ctxs = ctxsp.tile([P, QC], BF16, tag="ctxs", name="ctxs")
                for h in range(2):
                    _mark(nc, f"fin{g}_n{h}")
                    bc = pproj.tile([HD, QC], F32, tag="pp", name="pp")
                    nc.tensor.matmul(bc[:], ones64[:], recs[h][:], start=True, stop=True)
                    nc.vector.tensor_tensor(
                        out=ctxs[h * HD : (h + 1) * HD, :],
                        in0=cps[h][0:HD, :], in1=bc[:], op=MULT,
                    )
                    yield
                ob = outsp.tile([P, 4, 2 * QC], BF16, tag="ob", name="ob")
                for s in range(4):
                    _mark(nc, f"fin{g}_o{s}")
                    for oc in range(2):
                        po = pproj.tile([P, QC], F32, tag="pp", name="pp")
                        nc.tensor.matmul(
                            po[:], ctxs[:, s * P : (s + 1) * P],
                            wo_s[:, oc * QC : (oc + 1) * QC],
                            start=True, stop=True,
                        )
                        nc.vector.tensor_copy(ob[:, s, oc * QC : (oc + 1) * QC], po[:])
                    if s == 3:
                        # one strided DMA for the whole chunk's output
                        nc.sync.dma_start(out_v[:, g, :, :], ob[:])
                    yield

            fin_q = []

            def finq_iter():
                while fin_q:
                    try:
                        next(fin_q[0])
                        yield
                    except StopIteration:
                        fin_q.pop(0)

            # ---- schedule: proj(0); then attn(g) with proj(g+1) interleaved ----
            import os

            from itertools import chain

            reps = int(os.environ.get("BASS_KERNEL_REPS", "1"))
            load_weights(wq_s, wq)
            t0 = xtp.tile([P, KC, QC], BF16, tag="xt", name="xt")
            for half in range(2):
                nc.sync.dma_start(
                    t0[:, half * 4 : half * 4 + 4, :],
                    xT_v[:, half * 4 : half * 4 + 4, 0:QC],
                )
            xt_tiles[0] = t0
            load_weights(wk_s, wk)
            load_weights(wv_s, wv)
            load_xt(1)
            load_weights(wo_s, wo)
            # One flat pipeline over (rep, chunk): a single proj stream
            # (chunks 1..7 of the rep) drains one unit per j-tile slot into
            # whichever chunk has slots, with deadline forcing (Q before a
            # chunk starts, K before its diagonal tiles, V by its end). At
            # the rep wrap the next rep's proj(0) is a held side-iterator so
            # its q2T/k2T[0]/v2[0..3] overwrites land after chunk 7's reads.
            class Stream:
                def __init__(self, gens):
                    self.it = chain(*gens)
                    self.done = set()
                    self.dry = False

                def pull(self):
                    m = next(self.it, _SENT)
                    if m is _SENT:
                        self.dry = True
                        return False
                    self.done.add(m)
                    return True

                def force(self, marker):
                    while marker not in self.done and not self.dry:
                        self.pull()

            pend = []
            first = True
            for r in range(reps):
                if first:
                    load_xt(0)
                    load_xt(1)
                    stream = Stream([proj_units(g) for g in range(G)])
                    # emit chunk 0's q/k units up front; its v units
                    # interleave into attention(0) (AVs deferred so the PE
                    # order stays legal)
                    stream.force(("k", 0))
                else:
                    stream = next_stream
                for g in range(G):
                    wrap_iter = None
                    wrap_done = None
                    hold = 0
                    if g + 1 == G and r + 1 < reps:
                        load_xt(0)
                        load_xt(1)
                        # pre-run the next rep's proj(0..2) through chunk 7's
                        # exp-paced slots: early chunks have far fewer slots
                        # than proj units, so their proj work must come from
                        # here. Safe by pull order: proj(g')'s k2T[g']/v2
                        # writes land after chunk 7's tile-4g' reads/pops.
                        wrap_iter = chain(*[proj_units(gg) for gg in range(3)])
                        hold = 4
                        next_stream = Stream([proj_units(gg) for gg in range(3, G)])
                        # the wrap markers satisfy the next rep's chunk-0..2
                        # deadline checks
                        wrap_done = next_stream.done
                    pend = attention(
                        g, stream, pend, wrap_iter=wrap_iter, wrap_done=wrap_done,
                        defer_avs=first, hold_tiles=hold,
                        pre_force=("v3", g - 1) if g else None,
                    )
                    if wrap_iter is not None:
                        # chunk 7's v tiles must be emitted before its carried
                        # AVs pop in the next rep's attention(0), and the held
                        # proj(0..2) before the next rep reads its tiles
                        stream.force(("v3", G - 1))
                        for m in wrap_iter:
                            wrap_done.add(m)
                    first = False
            stream.force(("v3", G - 1))
            flush(pend)
            for _ in finq_iter():
                pass

    nc.compile()
    return nc


def _get_nc():
    if "nc" not in _CACHE:
        _CACHE["nc"] = build()
    return _CACHE["nc"]


def _get_runner():
    """jit(shard_map(bass_exec)) over 8 cores, built once and cached.

    Mirrors bass2jax.run_bass_via_pjrt's multi-core path minus donation, so
    the pre-zeroed output operands stay valid and every call after the first
    reuses the compiled executable.
    """
    if "runner" in _CACHE:
        return _CACHE["runner"]
    from concourse import bass2jax

    nc = _get_nc()
    bass2jax.install_neuronx_cc_hook()
    partition_name = nc.partition_id_tensor.name if nc.partition_id_tensor else None
    in_names, out_names, out_avals, zero_outs = [], [], [], []
    for alloc in nc.m.functions[0].allocations:
        if not isinstance(alloc, mybir.MemoryLocationSet):
            continue
        name = alloc.memorylocations[0].name
        if alloc.kind == "ExternalInput":
            if name != partition_name:
                in_names.append(name)
        elif alloc.kind == "ExternalOutput":
            shape = tuple(alloc.tensor_shape)
            dtype = mybir.dt.np(alloc.dtype)
            out_names.append(name)
            out_avals.append(jax.core.ShapedArray(shape, dtype))
            zero_outs.append(np.zeros(shape, dtype))
    n_params = len(in_names)
    all_in = in_names + out_names
    if partition_name is not None:
        all_in.append(partition_name)

    def _body(*args):
        operands = list(args)
        if partition_name is not None:
            operands.append(bass2jax.partition_id_tensor())
        return tuple(
            bass2jax._bass_exec_p.bind(
                *operands,
                out_avals=tuple(out_avals),
                in_names=tuple(all_in),
                out_names=tuple(out_names),
                lowering_input_output_aliases=(),
                sim_require_finite=True,
                sim_require_nnan=True,
                nc=nc,
            )
        )

    mesh = Mesh(np.asarray(jax.devices()[:8]), ("core",))
    spec = PartitionSpec("core")
    fn = jax.jit(
        shard_map(
            _body,
            mesh=mesh,
            in_specs=(spec,) * (n_params + len(out_names)),
            out_specs=(spec,) * len(out_names),
            check_rep=False,
        ),
        keep_unused=True,
    )
    sharding = NamedSharding(mesh, spec)
    zeros_dev = [
        jax.device_put(np.concatenate([z] * 8, axis=0), sharding) for z in zero_outs
    ]
    _CACHE["runner"] = (fn, in_names, out_names, out_avals, zeros_dev, sharding)
    return _CACHE["runner"]


def run_sharded(maps):
    """Run the SPMD kernel on 8 cores; returns list of per-core output dicts."""
    fn, in_names, out_names, out_avals, zeros_dev, sharding = _get_runner()
    concat_in = [
        jax.device_put(
            np.concatenate([np.asarray(maps[c][n]) for c in range(8)], axis=0), sharding
        )
        for n in in_names
    ]
    outs = fn(*concat_in, *zeros_dev)
    return [
        {
            n: np.asarray(outs[i]).reshape(8, *out_avals[i].shape)[c]
            for i, n in enumerate(out_names)
        }
        for c in range(8)
    ]


def make_in_maps(x, Wq, Wk, Wv, Wo):
    import ml_dtypes

    bf16 = ml_dtypes.bfloat16
    x = np.asarray(x, dtype=np.float32)
    xT = np.ascontiguousarray(x.reshape(N, D).T.astype(bf16))
    maps = []
    for c in range(8):
        sl = slice(c * P, (c + 1) * P)

        def _w(W):
            # [1024, 128] slice -> [p, kc*128+d] so SBUF gets lhsT chunks directly
            Wc = np.asarray(W, dtype=np.float32)[:, sl]
            return np.ascontiguousarray(
                Wc.reshape(KC, P, P).transpose(1, 0, 2).reshape(P, D).astype(bf16)
            )

        maps.append(
            {
                "xT": xT,
                "wq": _w(Wq),
                "wk": _w(Wk),
                "wv": _w(Wv),
                "wo": np.ascontiguousarray(
                    np.asarray(Wo, dtype=np.float32)[sl, :].astype(bf16)
                ),
            }
        )
    return maps


def kernel(x, Wq, Wk, Wv, Wo, bo):
    maps = make_in_maps(x, Wq, Wk, Wv, Wo)
    res = run_sharded(maps)
    acc = res[0]["out"].astype(np.float32)
    for c in range(1, 8):
        acc = acc + res[c]["out"]
    acc = acc + np.asarray(bo, dtype=np.float32)[None, :]
    return acc[None].astype(np.float32)
